# revision 41
# baseline (speedup 1.0000x reference)
"""Trainium2 Bass kernel for nn_MultiHeadAttnC (QANet-style self-attention).

Reference computation (per batch b):
    memory = w_mem @ queries[b]          # [2D, L]  (pointwise conv)
    query  = w_query @ queries[b]        # [D, L]
    K, V   = heads of memory             # H=8 heads, DH=16
    Q      = heads of query * DH^-0.5
    S      = Q @ K^T  (masked over kv)   # [H, L, L]
    out[b] = softmax(S) @ V  -> recombined to [D, L]

Strategy:
  - Data parallel: batch b -> NeuronCore b. Weights replicated. No collectives.
  - K-major ("transposed") attention per head: S^T[kv, q] = K^T.T @ Q^T with
    kv positions on PSUM partitions, computed with 4 heads concurrently via
    tensor-engine row tiling (contraction dim is only DH=16 -> 32-row groups).
  - Multiplicative 0/1 mask folds into a per-position validity vector that is
    multiplied into V and into an extra all-ones lhsT column, so exp(S) of a
    masked position contributes exactly 0 to both the numerator and the
    softmax denominator (exp never needs a mask bias, result is exact).
  - exp runs on the scalar engine straight out of PSUM (4 banks = N=2048 per
    instruction) into SBUF; the A/B head-group PSUM regions alternate so ACT
    stays 100% busy (it is the roofline engine: B*H*L^2 = 268M exps chip-wide).
  - AV matmul: out^T[dh, q] = [V | valid].T @ P^T via 32-column tiling (M=17:
    16 output channels + softmax denominator), accumulated across kv chunks
    on the vector engine in SBUF.
  - Final normalize: reciprocal of the denominator rows, DMA partition
    permute/broadcast to the output layout, one multiply. Output tile is
    already [D, L] — matches the reference's final transpose for free.
"""

import ml_dtypes
import numpy as np
from contextlib import ExitStack

import concourse.bass as bass
import concourse.tile as tile
from concourse import bacc, mybir
from concourse import bass_utils
from concourse.tile_rust import add_dep_helper as _add_dep_raw


def _add_dep(from_inst, to_inst, sync, reason):
    """add_dep_helper over BassInstruction wrappers (unwrap to mybir)."""
    def raw(i):
        return i.ins if isinstance(i, bass.BassInstruction) else i
    _add_dep_raw(raw(from_inst), raw(to_inst), sync=sync, reason=reason)

B, D, L, H, DH = 8, 128, 2048, 8, 16
f32 = mybir.dt.float32
f32r = mybir.dt.float32r
bf16 = mybir.dt.bfloat16
S_DT = bf16    # dtype of K/Q spread tiles (S^T matmul inputs)
AV_DT = bf16   # dtype of V tiles and exp output P (col tiling rejects f32r)
IN_DT = bf16   # dtype of DRAM inputs / projection matmul inputs (halves the
               # input DMA bytes; error stays ~1e-3 vs the 2e-2 gate)
QTILE = 512          # q columns per inner tile (one PSUM bank)
NJQ = L // QTILE     # 4

_program_cache: dict[int, "bacc.Bacc"] = {}


def _bcast_rows(t: bass.AP, f: int) -> bass.AP:
    """AP over tile t (free extent f) that reads partition rows
    {base, base+32, base+64, base+96}, each repeated 16x (DMA source only).
    Verified: dim0 steps flatly across partitions (pitch=f); the step-0 mid
    dim replicates; the DMA streams groups in order into a contiguous dest."""
    assert t.shape[0] == 1
    return bass.AP(tensor=t.tensor, offset=t.offset,
                   ap=[[32 * f, 4], [0, 16], [1, f]])


def _perm_rows(t: bass.AP, f: int) -> bass.AP:
    """AP over tile t (free extent f) that reads partition rows
    g*32+r for g in 0..3, r in 0..15 in row-major order (DMA source only):
    gathers the four 16-row head blocks of a 32-row-tiled accumulator into
    one contiguous 64-partition destination."""
    assert t.shape[0] == 1
    return bass.AP(tensor=t.tensor, offset=t.offset,
                   ap=[[32 * f, 4], [f, 16], [1, f]])


def _body(ctx, tc, qf_d, qkv_d, wq_d, wk_d, wv_d, val_d, em_d, out_d, n_kv):
    nc = tc.nc
    Lkv = n_kv * 128
    Exp = mybir.ActivationFunctionType.Exp

    consts = ctx.enter_context(tc.tile_pool(name="consts", bufs=1))

    # Inputs are bf16 and spread over the three DMA-capable queues
    # (sync/scalar/gpsimd, each ~80 GB/s) in the order the projections
    # consume them, so the PE never stalls on a late chunk.
    wq = consts.tile([D, 2, D], IN_DT, tag="wq")
    wk = consts.tile([D, 2, D], IN_DT, tag="wk")
    wv = consts.tile([D, D], IN_DT, tag="wv")
    qf = consts.tile([D, L], IN_DT, tag="qf")
    qkv = consts.tile([D, Lkv], IN_DT, tag="qkv")
    val = consts.tile([128, n_kv], f32, tag="val")
    # sync queue: X0 weights, then X1, then the later qf halves
    nc.sync.dma_start(out=wq[:, 0, :], in_=wq_d[0])
    nc.sync.dma_start(out=wk[:, 0, :], in_=wk_d[0])
    nc.sync.dma_start(out=wq[:, 1, :], in_=wq_d[1])
    nc.sync.dma_start(out=wk[:, 1, :], in_=wk_d[1])
    nc.sync.dma_start(out=wv, in_=wv_d)
    for j in (1, 3):
        nc.sync.dma_start(out=qf[:, j * QTILE:(j + 1) * QTILE],
                          in_=qf_d[:, j * QTILE:(j + 1) * QTILE])
    # scalar queue (idle pre-attention): first and third qf quarters
    for j in (0, 2):
        nc.scalar.dma_start(out=qf[:, j * QTILE:(j + 1) * QTILE],
                            in_=qf_d[:, j * QTILE:(j + 1) * QTILE])
    # gpsimd queue: kv-compacted queries front-to-back, then validity
    col = 0
    while col < Lkv:
        n = min(512, Lkv - col)
        nc.gpsimd.dma_start(out=qkv[:, col:col + n], in_=qkv_d[:, col:col + n])
        col += n
    nc.gpsimd.dma_start(out=val, in_=val_d)
    # one-hot row-permutation matrices for the final-pair epilogue, four
    # 128x128 blocks: [E0_d | E1_d | E0_v | E1_v] (d = denominator
    # broadcast, v = value permute; X halves accumulate in PSUM)
    emat = consts.tile([128, 512], f32r, tag="emat")
    nc.gpsimd.dma_start(out=emat, in_=em_d)
    ones8 = consts.tile([128, 8], f32, tag="ones8")
    nc.vector.memset(ones8, 1.0)

    q_sp = consts.tile([D, 2, L], S_DT, tag="q_sp")     # Q^T spread (pre-scaled)
    k_sp = consts.tile([D, 2, Lkv], S_DT, tag="k_sp")   # K^T spread
    v_sb = consts.tile([128, n_kv, H, DH + 1], AV_DT, tag="v_sb")
    out_sb = consts.tile([D, L], f32, tag="out_sb")

    # Two 4-bank PSUM regions (A/B head groups); all PSUM flows through them.
    # Allocated ONCE and reused: per-slot pool re-allocation made every new
    # tile's first write WAR-wait on ALL readers of the previous occupant
    # (whole-buffer granularity), which put the DVE drain on the critical
    # path ahead of the next S quad.  With a persistent tile, accesses are
    # tracked per byte-range, so S-quad banks 0-2 only wait on the exp read
    # and just the bank-3 matmul (emitted last) waits on the drain.
    spools = [
        ctx.enter_context(tc.tile_pool(name="spsA", bufs=1, space="PSUM")),
        ctx.enter_context(tc.tile_pool(name="spsB", bufs=1, space="PSUM")),
    ]
    sregs = [spools[X].tile([128, 4 * QTILE], f32, tag=f"s{X}", name=f"s{X}")
             for X in range(2)]

    def s_tile(X):
        return sregs[X]

    # HAM warmup: ~4.5us of back-to-back matmuls on scratch data while the
    # input DMAs land, so the PE clock-gate opens (1.2 -> 2.4 GHz) before the
    # projections and stays open (steady-state PE idle gaps are < the ~3.4us
    # re-throttle window).
    warm_in = consts.tile([128, 512], bf16, tag="warm_in")
    nc.vector.memset(warm_in, 0.0)
    wps = s_tile(0)
    for i in range(3):
        nc.tensor.matmul(wps[:, 0:512], lhsT=warm_in[:, 0:128],
                         rhs=warm_in, start=True, stop=True)

    # ---- projections ----
    # Emitted in input-arrival order (see the DMA queue layout above) so
    # the PE starts as soon as the first chunks land and never stalls.
    def q_proj(X, j):
        ps = s_tile(X)
        nc.tensor.matmul(ps[:, j * QTILE:(j + 1) * QTILE], lhsT=wq[:, X, :],
                         rhs=qf[:, j * QTILE:(j + 1) * QTILE],
                         start=True, stop=True)
        nc.vector.tensor_copy(out=q_sp[:, X, j * QTILE:(j + 1) * QTILE],
                              in_=ps[:, j * QTILE:(j + 1) * QTILE])

    def k_proj(X, m):
        ps = s_tile(X)
        col = m * 512
        n = min(512, Lkv - col)
        nc.tensor.matmul(ps[:, col:col + n], lhsT=wk[:, X, :],
                         rhs=qkv[:, col:col + n], start=True, stop=True)
        nc.vector.tensor_copy(out=k_sp[:, X, col:col + n],
                              in_=ps[:, col:col + n])

    n_km = -(-Lkv // 512)
    q_proj(0, 0)
    k_proj(0, 0)
    q_proj(1, 0)
    k_proj(1, 0)
    q_proj(0, 1)
    q_proj(1, 1)
    for m in range(1, n_km):
        k_proj(0, m)
        k_proj(1, m)
    for j in (2, 3):
        q_proj(0, j)
        q_proj(1, j)
    for c in range(n_kv):
        vp = s_tile(c % 2)
        nc.tensor.matmul(vp[:, 0:D], lhsT=qkv[:, c * 128:(c + 1) * 128],
                         rhs=wv, start=True, stop=True)
        # V columns scaled by validity (zeroes masked kv positions exactly)
        nc.vector.tensor_scalar_mul(
            v_sb[:, c, :, 0:DH],
            vp[:, 0:D].rearrange("p (h x) -> p h x", x=DH),
            val[:, c:c + 1])
        # ones column * validity -> softmax denominator source
        nc.vector.tensor_scalar_mul(
            v_sb[:, c, :, DH:DH + 1],
            ones8.rearrange("p (h x) -> p h x", x=1),
            val[:, c:c + 1])

    # ---- attention ----
    p_pool = ctx.enter_context(tc.tile_pool(name="p_pool", bufs=4))
    acc_pool = ctx.enter_context(tc.tile_pool(name="acc", bufs=2))
    misc = ctx.enter_context(tc.tile_pool(name="misc", bufs=2))

    # Four interleaved streams (2 q-chunks x 2 head-groups) over the two
    # 4-bank PSUM regions: each region is revisited only every other exp
    # slot, so its AV quad + DVE drain + next S quad all hide under the
    # other streams' exps and ACT runs back-to-back.
    #
    # Dependency surgery: Tile tracks the PSUM region at whole-tensor
    # granularity, so left alone the whole next S quad WAR-waits on the
    # DVE drain of bank 3, serializing exp->AV->drain->S->exp (measured
    # 2.29-2.39us/slot vs the 1.97us exp).  After each flush we wipe the
    # region's recorded accesses and wire the true bank-level deps by
    # hand: the S quad orders after the AV quad (same-engine, no
    # semaphore), waits on the region's previous exp (which read banks
    # 0-3), and only the bank-3 matmul (emitted last) waits on the drain.
    last_exp = [None, None]    # per region: exp instruction of prev visit
    last_drain = [None, None]  # per region: drain instruction of prev visit
    last_avmm = [None, None]   # per region: last AV matmul of prev visit
    deferred = []  # pair-0 epilogue thunks, drained one per round in pair 1
                   # so their DVE ops don't queue ahead of critical drains
    for jp in range(NJQ // 2):
        streams = [(2 * jp, 0), (2 * jp, 1), (2 * jp + 1, 0), (2 * jp + 1, 1)]
        # f32r so the final-pair epilogue matmuls may consume them (the
        # BIR verifier requires f32r inputs to be produced f32r-rounded)
        accs = {s: acc_pool.tile([128, QTILE], f32r, tag=f"acc{i}",
                                 name=f"acc{i}")
                for i, s in enumerate(streams)}
        pend = [None, None]  # per PSUM region: (p_sb, c, stream)

        def flush_av(R):
            p_sb, c, s = pend[R]
            sp = sregs[R]
            X = s[1]
            # AV quad into (consumed) bank 3 of that iteration's PSUM region:
            # out^T[dh,q] += [V|valid].T @ P^T  (col tiling, M=17)
            bk = slice(3 * QTILE, 4 * QTILE)
            bk = slice(3 * QTILE, 4 * QTILE)
            for g in range(4):
                h = X * 4 + g
                mm = nc.tensor.matmul(
                    sp[g * 32:g * 32 + DH + 1, bk],
                    lhsT=v_sb[:, c, h, :],
                    rhs=p_sb[:, g * QTILE:(g + 1) * QTILE],
                    start=True, stop=True, tile_position=(0, g * 32))
            if c == 0:
                dr = nc.vector.tensor_copy(out=accs[s], in_=sp[:, bk])
            else:
                dr = nc.vector.tensor_add(out=accs[s], in0=accs[s],
                                          in1=sp[:, bk])
            last_drain[R] = dr
            last_avmm[R] = mm
            pend[R] = None

        for c in range(n_kv):
            if deferred:
                deferred.pop(0)()
            for jq, X in streams:
                flushed = pend[X] is not None
                if flushed:
                    flush_av(X)
                    tc.dep_state.clear_tensor_accesses(sregs[X].tensor.name)
                qs = slice(jq * QTILE, (jq + 1) * QTILE)
                sp = s_tile(X)
                # S^T quad: 4 heads concurrently (row tiling, K=32 incl. 0s)
                mms = []
                for g in range(4):
                    mms.append(nc.tensor.matmul(
                        sp[:, g * QTILE:(g + 1) * QTILE],
                        lhsT=k_sp[g * 32:(g + 1) * 32, X, c * 128:(c + 1) * 128],
                        rhs=q_sp[g * 32:(g + 1) * 32, X, qs],
                        start=True, stop=True, tile_position=(g * 32, 0)))
                if flushed:
                    _add_dep(mms[0], last_exp[X], sync=True,
                             reason="S quad WAR on region's prev exp")
                    _add_dep(mms[0], last_avmm[X], sync=False,
                             reason="S quad after AV quad (PE order)")
                    _add_dep(mms[3], last_drain[X], sync=True,
                             reason="S bank-3 WAR on drain")
                p_sb = p_pool.tile([128, 4 * QTILE], AV_DT, tag="p")
                last_exp[X] = nc.scalar.activation(out=p_sb, in_=sp, func=Exp)
                pend[X] = (p_sb, c, (jq, X))
        for R in range(2):
            flush_av(R)
        while deferred and jp == 1:
            deferred.pop(0)()

        # ---- per-pair epilogue: normalize + assemble output layout ----
        for jq in (2 * jp, 2 * jp + 1):
            qs = slice(jq * QTILE, (jq + 1) * QTILE)
            if jp == 0:
                # mid-kernel: DMA-based permute/broadcast on the sync queue
                # (PE and PSUM are busy with the next pair's steady state).
                # Emission is deferred into pair 1's rounds; the multiply
                # runs on the Pool engine to keep DVE free for drains.
                def _ep_recs(X, jq=jq, a=accs):
                    recs = misc.tile([128, QTILE], f32, tag=f"rec{X}",
                                     name=f"rec{X}")
                    nc.vector.reciprocal_approx_fast(
                        out=recs, in_=a[(jq, X)].bitcast(f32))
                    return recs
                def _ep_dmas(st, jq=jq, a=accs):
                    xt = misc.tile([128, QTILE], f32, tag="xt")
                    rb = misc.tile([128, QTILE], f32, tag="rb")
                    for X in range(2):
                        for g in range(4):
                            h = X * 4 + g
                            nc.sync.dma_start(
                                out=xt[h * DH:(h + 1) * DH, :],
                                in_=a[(jq, X)][g * 32:g * 32 + DH, :].bitcast(f32))
                        nc.sync.dma_start(
                            out=rb[X * 64:(X + 1) * 64, :],
                            in_=_bcast_rows(st[X][DH:DH + 1, :], QTILE))
                    return xt, rb
                def _ep_mul(st, qs=qs):
                    xt, rb = st
                    nc.vector.tensor_mul(out=out_sb[:, qs], in0=xt, in1=rb)
                    nc.sync.dma_start(out=out_d[:, qs], in_=out_sb[:, qs])
                state = {}
                deferred.append(
                    lambda f=_ep_recs, s=state: s.__setitem__(0, f(0)))
                deferred.append(
                    lambda f=_ep_recs, s=state: s.__setitem__(1, f(1)))
                deferred.append(
                    lambda f=_ep_dmas, s=state: s.__setitem__(
                        'd', f((s[0], s[1]))))
                deferred.append(lambda f=_ep_mul, s=state: f(s['d']))
            else:
                # final pair: PE and PSUM are free once the last exps
                # retire, so permute values and broadcast denominators with
                # one-hot matmuls instead of slow many-descriptor DMAs:
                # out = (Sum_X Ev_X^T @ acc_X) * recip(Sum_X Ed_X^T @ acc_X)
                # (full-width M=128 accumulating matmuls; col tiling would
                # reject f32r)
                sp = sregs[jq - 2]
                for X in range(2):
                    nc.tensor.matmul(
                        sp[:, 0:QTILE], lhsT=emat[:, X * 128:(X + 1) * 128],
                        rhs=accs[(jq, X)], start=(X == 0), stop=(X == 1))
                for X in range(2):
                    nc.tensor.matmul(
                        sp[:, QTILE:2 * QTILE],
                        lhsT=emat[:, 256 + X * 128:256 + (X + 1) * 128],
                        rhs=accs[(jq, X)], start=(X == 0), stop=(X == 1))
                rec2 = misc.tile([128, QTILE], f32, tag="rec2")
                nc.vector.reciprocal_approx_fast(out=rec2, in_=sp[:, 0:QTILE])
                nc.vector.tensor_mul(out=out_sb[:, qs],
                                     in0=sp[:, QTILE:2 * QTILE], in1=rec2)
                nc.sync.dma_start(out=out_d[:, qs], in_=out_sb[:, qs])


def _build(n_kv: int) -> "bacc.Bacc":
    Lkv = n_kv * 128
    nc = bacc.Bacc("TRN2", target_bir_lowering=False, debug=False,
                   enable_asserts=True, num_devices=B)
    qf_d = nc.dram_tensor("q_full", [D, L], IN_DT, kind="ExternalInput").ap()
    qkv_d = nc.dram_tensor("q_kv", [D, Lkv], IN_DT, kind="ExternalInput").ap()
    wq_d = nc.dram_tensor("wq_sp", [2, D, D], IN_DT, kind="ExternalInput").ap()
    wk_d = nc.dram_tensor("wk_sp", [2, D, D], IN_DT, kind="ExternalInput").ap()
    wv_d = nc.dram_tensor("wv_t", [D, D], IN_DT, kind="ExternalInput").ap()
    val_d = nc.dram_tensor("valid", [128, n_kv], f32, kind="ExternalInput").ap()
    em_d = nc.dram_tensor("emat", [128, 512], f32r,
                          kind="ExternalInput").ap()
    out_d = nc.dram_tensor("out", [D, L], f32, kind="ExternalOutput").ap()

    with tile.TileContext(nc) as tc, ExitStack() as ctx:
        _body(ctx, tc, qf_d, qkv_d, wq_d, wk_d, wv_d, val_d, em_d, out_d, n_kv)
    nc.compile()
    return nc


def _prep_weights(w_mem: np.ndarray, w_query: np.ndarray):
    """Spread head weights into 32-row tile groups (rows 16:32 zero) and
    pre-transpose for use as matmul lhsT. Q gets the DH^-0.5 scale.
    Cast bf16 (matmul input dtype; DMA'd as-is)."""
    wq_sp = np.zeros((2, D, D), np.float32)
    wk_sp = np.zeros((2, D, D), np.float32)
    scale = np.float32(DH ** -0.5)
    for X in range(2):
        for g in range(4):
            h = 4 * X + g
            wq_sp[X][:, 32 * g:32 * g + DH] = (w_query[DH * h:DH * (h + 1), :] * scale).T
            wk_sp[X][:, 32 * g:32 * g + DH] = w_mem[DH * h:DH * (h + 1), :].T
    wv_t = np.ascontiguousarray(w_mem[D:2 * D, :].T)
    bf = ml_dtypes.bfloat16
    return wq_sp.astype(bf), wk_sp.astype(bf), wv_t.astype(bf)


def _emat() -> np.ndarray:
    """One-hot row maps for the final-pair epilogue matmuls, four 128x128
    lhsT blocks [E0_d | E1_d | E0_v | E1_v]:
    Ed_X[g*32+16, X*64+g*16+r] = 1 for r<16 (denominator broadcast)
    Ev_X[g*32+r,  X*64+g*16+r] = 1          (value row permute)
    The X=0/1 matmuls accumulate into one PSUM bank, each filling its own
    64-partition half."""
    e = np.zeros((128, 512), np.float32)
    for X in range(2):
        for g in range(4):
            for r in range(DH):
                e[g * 32 + DH, X * 128 + X * 64 + g * 16 + r] = 1.0
                e[g * 32 + r, 256 + X * 128 + X * 64 + g * 16 + r] = 1.0
    return e


COMPACT_KV = True  # drop masked kv positions host-side (exact: they contribute
                   # exp(-1e30)=0 to softmax numerator and denominator alike)


def prepare(queries: np.ndarray, mask: np.ndarray, w_mem: np.ndarray,
            w_query: np.ndarray):
    """Build (compiled program, per-core input maps)."""
    assert queries.shape == (B, D, L) and mask.shape == (B, L)
    maskf = mask.astype(np.float32)
    kept = [np.nonzero(maskf[b] > 0.0)[0] for b in range(B)]
    if COMPACT_KV and all(len(k) > 0 for k in kept):
        n_kv = max(1, -(-max(len(k) for k in kept) // 128))
    else:
        n_kv = L // 128
        kept = None
    Lkv = n_kv * 128

    nc = _program_cache.get(n_kv)
    if nc is None:
        nc = _program_cache[n_kv] = _build(n_kv)

    wq_sp, wk_sp, wv_t = _prep_weights(
        w_mem.astype(np.float32), w_query.astype(np.float32))
    em = _emat()

    bf = ml_dtypes.bfloat16
    in_maps = []
    for b in range(B):
        qb = np.ascontiguousarray(queries[b], dtype=np.float32)
        if kept is not None:
            idx = kept[b]
            qkv = np.zeros((D, Lkv), np.float32)
            qkv[:, :len(idx)] = qb[:, idx]
            val = np.zeros(Lkv, np.float32)
            val[:len(idx)] = 1.0
        else:
            qkv = qb
            val = maskf[b]
        in_maps.append({
            "q_full": np.ascontiguousarray(qb.astype(bf)),
            "q_kv": np.ascontiguousarray(qkv.astype(bf)),
            "wq_sp": wq_sp,
            "wk_sp": wk_sp,
            "wv_t": wv_t,
            "valid": np.ascontiguousarray(val.reshape(n_kv, 128).T),
            "emat": em,
        })
    return nc, in_maps


def kernel(queries: np.ndarray, mask: np.ndarray, w_mem: np.ndarray,
           w_query: np.ndarray) -> np.ndarray:
    nc, in_maps = prepare(queries, mask, w_mem, w_query)
    res = bass_utils.run_bass_kernel_spmd(nc, in_maps, core_ids=list(range(B)))
    return np.stack([res.results[b]["out"] for b in range(B)]).astype(np.float32)



# revision 43
# speedup vs baseline: 1.0057x; 1.0057x over previous
"""Trainium2 Bass kernel for nn_MultiHeadAttnC (QANet-style self-attention).

Reference computation (per batch b):
    memory = w_mem @ queries[b]          # [2D, L]  (pointwise conv)
    query  = w_query @ queries[b]        # [D, L]
    K, V   = heads of memory             # H=8 heads, DH=16
    Q      = heads of query * DH^-0.5
    S      = Q @ K^T  (masked over kv)   # [H, L, L]
    out[b] = softmax(S) @ V  -> recombined to [D, L]

Strategy:
  - Data parallel: batch b -> NeuronCore b. Weights replicated. No collectives.
  - All inputs bf16 host-side (halves input DMA bytes, doubles matmul rate
    under the HAM 50%-duty throttle; ~4e-3 median rel err vs the 2e-2 gate),
    spread over the three DMA-capable queues in consumption order.
  - K-major ("transposed") attention per head: S^T[kv, q] = K^T.T @ Q^T with
    kv positions on PSUM partitions, computed with 4 heads concurrently via
    tensor-engine row tiling (contraction dim is only DH=16 -> 32-row groups).
  - Multiplicative 0/1 mask folds into a per-position validity vector that is
    multiplied into V and into an extra all-ones lhsT column, so exp(S) of a
    masked position contributes exactly 0 to both the numerator and the
    softmax denominator (exp never needs a mask bias, result is exact).
  - exp runs on the scalar engine straight out of PSUM (4 banks = N=2048 per
    instruction) into SBUF; the A/B head-group PSUM regions alternate so ACT
    stays ~94% busy (it is the roofline engine: B*H*L^2 = 268M exps chip-wide,
    104 x 1.97us per core after kv compaction).
  - AV matmul: out^T[dh, q] = [V | valid].T @ P^T via 32-column tiling (M=17:
    16 output channels + softmax denominator), accumulated across kv chunks
    on the vector engine in SBUF (f32r so the epilogue matmuls may read it).
  - Tile tracks PSUM at whole-tensor granularity, which would serialize
    exp->AV->drain->S->exp; persistent region tiles + clear_tensor_accesses
    + hand-wired deps let S banks 0-2 run under the exp and only the bank-3
    matmul wait on the drain (2.39 -> 2.11us/slot).
  - Mid-kernel epilogue (first q-pair) normalizes via DMA permute/broadcast
    on the sync queue, emitted as thunks spread over the next pair's rounds
    so its DVE ops never queue ahead of critical drains.  The final pair
    instead uses one-hot permutation matmuls on the then-idle PE and PSUM.
"""

import ml_dtypes
import numpy as np
from contextlib import ExitStack

import concourse.bass as bass
import concourse.tile as tile
from concourse import bacc, mybir
from concourse import bass_utils
from concourse.tile_rust import add_dep_helper as _add_dep_raw


def _add_dep(from_inst, to_inst, sync, reason):
    """add_dep_helper over BassInstruction wrappers (unwrap to mybir)."""
    def raw(i):
        return i.ins if isinstance(i, bass.BassInstruction) else i
    _add_dep_raw(raw(from_inst), raw(to_inst), sync=sync, reason=reason)

B, D, L, H, DH = 8, 128, 2048, 8, 16
f32 = mybir.dt.float32
f32r = mybir.dt.float32r
bf16 = mybir.dt.bfloat16
S_DT = bf16    # dtype of K/Q spread tiles (S^T matmul inputs)
AV_DT = bf16   # dtype of V tiles and exp output P (col tiling rejects f32r)
IN_DT = bf16   # dtype of DRAM inputs / projection matmul inputs (halves the
               # input DMA bytes; error stays ~1e-3 vs the 2e-2 gate)
QTILE = 512          # q columns per inner tile (one PSUM bank)
NJQ = L // QTILE     # 4

_program_cache: dict[int, "bacc.Bacc"] = {}


def _bcast_rows(t: bass.AP, f: int) -> bass.AP:
    """AP over tile t (free extent f) that reads partition rows
    {base, base+32, base+64, base+96}, each repeated 16x (DMA source only).
    Verified: dim0 steps flatly across partitions (pitch=f); the step-0 mid
    dim replicates; the DMA streams groups in order into a contiguous dest."""
    assert t.shape[0] == 1
    return bass.AP(tensor=t.tensor, offset=t.offset,
                   ap=[[32 * f, 4], [0, 16], [1, f]])


def _perm_rows(t: bass.AP, f: int) -> bass.AP:
    """AP over tile t (free extent f) that reads partition rows
    g*32+r for g in 0..3, r in 0..15 in row-major order (DMA source only):
    gathers the four 16-row head blocks of a 32-row-tiled accumulator into
    one contiguous 64-partition destination."""
    assert t.shape[0] == 1
    return bass.AP(tensor=t.tensor, offset=t.offset,
                   ap=[[32 * f, 4], [f, 16], [1, f]])


def _body(ctx, tc, qf_d, qkv_d, wq_d, wk_d, wv_d, val_d, em_d, out_d, n_kv):
    nc = tc.nc
    Lkv = n_kv * 128
    Exp = mybir.ActivationFunctionType.Exp

    consts = ctx.enter_context(tc.tile_pool(name="consts", bufs=1))

    # Inputs are bf16 and spread over the three DMA-capable queues
    # (sync/scalar/gpsimd, each ~80 GB/s) in the order the projections
    # consume them, so the PE never stalls on a late chunk.
    wq = consts.tile([D, 2, D], IN_DT, tag="wq")
    wk = consts.tile([D, 2, D], IN_DT, tag="wk")
    wv = consts.tile([D, D], IN_DT, tag="wv")
    qf = consts.tile([D, L], IN_DT, tag="qf")
    qkv = consts.tile([D, Lkv], IN_DT, tag="qkv")
    val = consts.tile([128, n_kv], f32, tag="val")
    # sync queue: X0 weights, then X1, then the later qf halves
    nc.sync.dma_start(out=wq[:, 0, :], in_=wq_d[0])
    nc.sync.dma_start(out=wk[:, 0, :], in_=wk_d[0])
    nc.sync.dma_start(out=wq[:, 1, :], in_=wq_d[1])
    nc.sync.dma_start(out=wk[:, 1, :], in_=wk_d[1])
    nc.sync.dma_start(out=wv, in_=wv_d)
    for j in (1, 3):
        nc.sync.dma_start(out=qf[:, j * QTILE:(j + 1) * QTILE],
                          in_=qf_d[:, j * QTILE:(j + 1) * QTILE])
    # scalar queue (idle pre-attention): first and third qf quarters
    for j in (0, 2):
        nc.scalar.dma_start(out=qf[:, j * QTILE:(j + 1) * QTILE],
                            in_=qf_d[:, j * QTILE:(j + 1) * QTILE])
    # gpsimd queue: kv-compacted queries front-to-back, then validity
    col = 0
    while col < Lkv:
        n = min(512, Lkv - col)
        nc.gpsimd.dma_start(out=qkv[:, col:col + n], in_=qkv_d[:, col:col + n])
        col += n
    nc.gpsimd.dma_start(out=val, in_=val_d)
    # one-hot row-permutation matrices for the final-pair epilogue, four
    # 128x128 blocks: [E0_d | E1_d | E0_v | E1_v] (d = denominator
    # broadcast, v = value permute; X halves accumulate in PSUM)
    emat = consts.tile([128, 512], f32r, tag="emat")
    nc.gpsimd.dma_start(out=emat, in_=em_d)
    ones8 = consts.tile([128, 8], f32, tag="ones8")
    nc.vector.memset(ones8, 1.0)

    q_sp = consts.tile([D, 2, L], S_DT, tag="q_sp")     # Q^T spread (pre-scaled)
    k_sp = consts.tile([D, 2, Lkv], S_DT, tag="k_sp")   # K^T spread
    v_sb = consts.tile([128, n_kv, H, DH + 1], AV_DT, tag="v_sb")
    out_sb = consts.tile([D, L], f32, tag="out_sb")

    # Two 4-bank PSUM regions (A/B head groups); all PSUM flows through them.
    # Allocated ONCE and reused: per-slot pool re-allocation made every new
    # tile's first write WAR-wait on ALL readers of the previous occupant
    # (whole-buffer granularity), which put the DVE drain on the critical
    # path ahead of the next S quad.  With a persistent tile, accesses are
    # tracked per byte-range, so S-quad banks 0-2 only wait on the exp read
    # and just the bank-3 matmul (emitted last) waits on the drain.
    spools = [
        ctx.enter_context(tc.tile_pool(name="spsA", bufs=1, space="PSUM")),
        ctx.enter_context(tc.tile_pool(name="spsB", bufs=1, space="PSUM")),
    ]
    sregs = [spools[X].tile([128, 4 * QTILE], f32, tag=f"s{X}", name=f"s{X}")
             for X in range(2)]

    def s_tile(X):
        return sregs[X]

    # HAM warmup: ~4.5us of back-to-back matmuls on scratch data while the
    # input DMAs land, so the PE clock-gate opens (1.2 -> 2.4 GHz) before the
    # projections and stays open (steady-state PE idle gaps are < the ~3.4us
    # re-throttle window).
    warm_in = consts.tile([128, 512], bf16, tag="warm_in")
    nc.vector.memset(warm_in, 0.0)
    wps = s_tile(0)
    for i in range(3):
        nc.tensor.matmul(wps[:, 0:512], lhsT=warm_in[:, 0:128],
                         rhs=warm_in, start=True, stop=True)

    # ---- projections ----
    # Emitted in input-arrival order (see the DMA queue layout above) so
    # the PE starts as soon as the first chunks land and never stalls.
    def q_proj(X, j):
        ps = s_tile(X)
        nc.tensor.matmul(ps[:, j * QTILE:(j + 1) * QTILE], lhsT=wq[:, X, :],
                         rhs=qf[:, j * QTILE:(j + 1) * QTILE],
                         start=True, stop=True)
        nc.vector.tensor_copy(out=q_sp[:, X, j * QTILE:(j + 1) * QTILE],
                              in_=ps[:, j * QTILE:(j + 1) * QTILE])

    def k_proj(X, m):
        ps = s_tile(X)
        col = m * 512
        n = min(512, Lkv - col)
        nc.tensor.matmul(ps[:, col:col + n], lhsT=wk[:, X, :],
                         rhs=qkv[:, col:col + n], start=True, stop=True)
        nc.vector.tensor_copy(out=k_sp[:, X, col:col + n],
                              in_=ps[:, col:col + n])

    n_km = -(-Lkv // 512)
    q_proj(0, 0)
    k_proj(0, 0)
    q_proj(1, 0)
    k_proj(1, 0)
    q_proj(0, 1)
    q_proj(1, 1)
    for m in range(1, n_km):
        k_proj(0, m)
        k_proj(1, m)
    # q-tiles 2,3 are only needed by the second jq pair ~130us in; emitted
    # at the pair transition (see the jp loop) to shorten the prologue.
    for c in range(n_kv):
        vp = s_tile(c % 2)
        nc.tensor.matmul(vp[:, 0:D], lhsT=qkv[:, c * 128:(c + 1) * 128],
                         rhs=wv, start=True, stop=True)
        # V columns scaled by validity (zeroes masked kv positions exactly)
        nc.vector.tensor_scalar_mul(
            v_sb[:, c, :, 0:DH],
            vp[:, 0:D].rearrange("p (h x) -> p h x", x=DH),
            val[:, c:c + 1])
        # ones column * validity -> softmax denominator source
        nc.vector.tensor_scalar_mul(
            v_sb[:, c, :, DH:DH + 1],
            ones8.rearrange("p (h x) -> p h x", x=1),
            val[:, c:c + 1])

    # ---- attention ----
    p_pool = ctx.enter_context(tc.tile_pool(name="p_pool", bufs=4))
    acc_pool = ctx.enter_context(tc.tile_pool(name="acc", bufs=2))
    misc = ctx.enter_context(tc.tile_pool(name="misc", bufs=2))

    # Four interleaved streams (2 q-chunks x 2 head-groups) over the two
    # 4-bank PSUM regions: each region is revisited only every other exp
    # slot, so its AV quad + DVE drain + next S quad all hide under the
    # other streams' exps and ACT runs back-to-back.
    #
    # Dependency surgery: Tile tracks the PSUM region at whole-tensor
    # granularity, so left alone the whole next S quad WAR-waits on the
    # DVE drain of bank 3, serializing exp->AV->drain->S->exp (measured
    # 2.29-2.39us/slot vs the 1.97us exp).  After each flush we wipe the
    # region's recorded accesses and wire the true bank-level deps by
    # hand: the S quad orders after the AV quad (same-engine, no
    # semaphore), waits on the region's previous exp (which read banks
    # 0-3), and only the bank-3 matmul (emitted last) waits on the drain.
    last_exp = [None, None]    # per region: exp instruction of prev visit
    last_drain = [None, None]  # per region: drain instruction of prev visit
    last_avmm = [None, None]   # per region: last AV matmul of prev visit
    deferred = []  # pair-0 epilogue thunks, drained one per round in pair 1
                   # so their DVE ops don't queue ahead of critical drains
    for jp in range(NJQ // 2):
        streams = [(2 * jp, 0), (2 * jp, 1), (2 * jp + 1, 0), (2 * jp + 1, 1)]
        # f32r so the final-pair epilogue matmuls may consume them (the
        # BIR verifier requires f32r inputs to be produced f32r-rounded)
        accs = {s: acc_pool.tile([128, QTILE], f32r, tag=f"acc{i}",
                                 name=f"acc{i}")
                for i, s in enumerate(streams)}
        pend = [None, None]  # per PSUM region: (p_sb, c, stream)

        def flush_av(R):
            p_sb, c, s = pend[R]
            sp = sregs[R]
            X = s[1]
            # AV quad into (consumed) bank 3 of that iteration's PSUM region:
            # out^T[dh,q] += [V|valid].T @ P^T  (col tiling, M=17)
            bk = slice(3 * QTILE, 4 * QTILE)
            bk = slice(3 * QTILE, 4 * QTILE)
            for g in range(4):
                h = X * 4 + g
                mm = nc.tensor.matmul(
                    sp[g * 32:g * 32 + DH + 1, bk],
                    lhsT=v_sb[:, c, h, :],
                    rhs=p_sb[:, g * QTILE:(g + 1) * QTILE],
                    start=True, stop=True, tile_position=(0, g * 32))
            if c == 0:
                dr = nc.vector.tensor_copy(out=accs[s], in_=sp[:, bk])
            else:
                dr = nc.vector.tensor_add(out=accs[s], in0=accs[s],
                                          in1=sp[:, bk])
            last_drain[R] = dr
            last_avmm[R] = mm
            pend[R] = None

        for c in range(n_kv):
            if deferred:
                deferred.pop(0)()
            for jq, X in streams:
                flushed = pend[X] is not None
                if flushed:
                    flush_av(X)
                    tc.dep_state.clear_tensor_accesses(sregs[X].tensor.name)
                qs = slice(jq * QTILE, (jq + 1) * QTILE)
                sp = s_tile(X)
                # S^T quad: 4 heads concurrently (row tiling, K=32 incl. 0s)
                mms = []
                for g in range(4):
                    mms.append(nc.tensor.matmul(
                        sp[:, g * QTILE:(g + 1) * QTILE],
                        lhsT=k_sp[g * 32:(g + 1) * 32, X, c * 128:(c + 1) * 128],
                        rhs=q_sp[g * 32:(g + 1) * 32, X, qs],
                        start=True, stop=True, tile_position=(g * 32, 0)))
                if flushed:
                    _add_dep(mms[0], last_exp[X], sync=True,
                             reason="S quad WAR on region's prev exp")
                    _add_dep(mms[0], last_avmm[X], sync=False,
                             reason="S quad after AV quad (PE order)")
                    _add_dep(mms[3], last_drain[X], sync=True,
                             reason="S bank-3 WAR on drain")
                p_sb = p_pool.tile([128, 4 * QTILE], AV_DT, tag="p")
                last_exp[X] = nc.scalar.activation(out=p_sb, in_=sp, func=Exp)
                pend[X] = (p_sb, c, (jq, X))
        for R in range(2):
            flush_av(R)
        if jp == 0:
            # deferred q-tile 2,3 projections: the regions are briefly free
            # between the pairs and the matmuls+casts hide under the final
            # exps of pair 0.
            for j in (2, 3):
                q_proj(0, j)
                q_proj(1, j)
        while deferred and jp == 1:
            deferred.pop(0)()

        # ---- per-pair epilogue: normalize + assemble output layout ----
        for jq in (2 * jp, 2 * jp + 1):
            qs = slice(jq * QTILE, (jq + 1) * QTILE)
            if jp == 0:
                # mid-kernel: DMA-based permute/broadcast on the sync queue
                # (PE and PSUM are busy with the next pair's steady state).
                # Emission is deferred into pair 1's rounds; the multiply
                # runs on the Pool engine to keep DVE free for drains.
                def _ep_recs(X, jq=jq, a=accs):
                    recs = misc.tile([128, QTILE], f32, tag=f"rec{X}",
                                     name=f"rec{X}")
                    nc.vector.reciprocal_approx_fast(
                        out=recs, in_=a[(jq, X)].bitcast(f32))
                    return recs
                def _ep_dmas(st, jq=jq, a=accs):
                    xt = misc.tile([128, QTILE], f32, tag="xt")
                    rb = misc.tile([128, QTILE], f32, tag="rb")
                    for X in range(2):
                        for g in range(4):
                            h = X * 4 + g
                            nc.sync.dma_start(
                                out=xt[h * DH:(h + 1) * DH, :],
                                in_=a[(jq, X)][g * 32:g * 32 + DH, :].bitcast(f32))
                        nc.sync.dma_start(
                            out=rb[X * 64:(X + 1) * 64, :],
                            in_=_bcast_rows(st[X][DH:DH + 1, :], QTILE))
                    return xt, rb
                def _ep_mul(st, qs=qs):
                    xt, rb = st
                    nc.vector.tensor_mul(out=out_sb[:, qs], in0=xt, in1=rb)
                    nc.sync.dma_start(out=out_d[:, qs], in_=out_sb[:, qs])
                state = {}
                deferred.append(
                    lambda f=_ep_recs, s=state: s.__setitem__(0, f(0)))
                deferred.append(
                    lambda f=_ep_recs, s=state: s.__setitem__(1, f(1)))
                deferred.append(
                    lambda f=_ep_dmas, s=state: s.__setitem__(
                        'd', f((s[0], s[1]))))
                deferred.append(lambda f=_ep_mul, s=state: f(s['d']))
            else:
                # final pair: PE and PSUM are free once the last exps
                # retire, so permute values and broadcast denominators with
                # one-hot matmuls instead of slow many-descriptor DMAs:
                # out = (Sum_X Ev_X^T @ acc_X) * recip(Sum_X Ed_X^T @ acc_X)
                # (full-width M=128 accumulating matmuls; col tiling would
                # reject f32r)
                sp = sregs[jq - 2]
                for X in range(2):
                    nc.tensor.matmul(
                        sp[:, 0:QTILE], lhsT=emat[:, X * 128:(X + 1) * 128],
                        rhs=accs[(jq, X)], start=(X == 0), stop=(X == 1))
                for X in range(2):
                    nc.tensor.matmul(
                        sp[:, QTILE:2 * QTILE],
                        lhsT=emat[:, 256 + X * 128:256 + (X + 1) * 128],
                        rhs=accs[(jq, X)], start=(X == 0), stop=(X == 1))
                rec2 = misc.tile([128, QTILE], f32, tag="rec2")
                nc.vector.reciprocal_approx_fast(out=rec2, in_=sp[:, 0:QTILE])
                nc.vector.tensor_mul(out=out_sb[:, qs],
                                     in0=sp[:, QTILE:2 * QTILE], in1=rec2)
                nc.sync.dma_start(out=out_d[:, qs], in_=out_sb[:, qs])


def _build(n_kv: int) -> "bacc.Bacc":
    Lkv = n_kv * 128
    nc = bacc.Bacc("TRN2", target_bir_lowering=False, debug=False,
                   enable_asserts=True, num_devices=B)
    qf_d = nc.dram_tensor("q_full", [D, L], IN_DT, kind="ExternalInput").ap()
    qkv_d = nc.dram_tensor("q_kv", [D, Lkv], IN_DT, kind="ExternalInput").ap()
    wq_d = nc.dram_tensor("wq_sp", [2, D, D], IN_DT, kind="ExternalInput").ap()
    wk_d = nc.dram_tensor("wk_sp", [2, D, D], IN_DT, kind="ExternalInput").ap()
    wv_d = nc.dram_tensor("wv_t", [D, D], IN_DT, kind="ExternalInput").ap()
    val_d = nc.dram_tensor("valid", [128, n_kv], f32, kind="ExternalInput").ap()
    em_d = nc.dram_tensor("emat", [128, 512], f32r,
                          kind="ExternalInput").ap()
    out_d = nc.dram_tensor("out", [D, L], f32, kind="ExternalOutput").ap()

    with tile.TileContext(nc) as tc, ExitStack() as ctx:
        _body(ctx, tc, qf_d, qkv_d, wq_d, wk_d, wv_d, val_d, em_d, out_d, n_kv)
    nc.compile()
    return nc


def _prep_weights(w_mem: np.ndarray, w_query: np.ndarray):
    """Spread head weights into 32-row tile groups (rows 16:32 zero) and
    pre-transpose for use as matmul lhsT. Q gets the DH^-0.5 scale.
    Cast bf16 (matmul input dtype; DMA'd as-is)."""
    wq_sp = np.zeros((2, D, D), np.float32)
    wk_sp = np.zeros((2, D, D), np.float32)
    scale = np.float32(DH ** -0.5)
    for X in range(2):
        for g in range(4):
            h = 4 * X + g
            wq_sp[X][:, 32 * g:32 * g + DH] = (w_query[DH * h:DH * (h + 1), :] * scale).T
            wk_sp[X][:, 32 * g:32 * g + DH] = w_mem[DH * h:DH * (h + 1), :].T
    wv_t = np.ascontiguousarray(w_mem[D:2 * D, :].T)
    bf = ml_dtypes.bfloat16
    return wq_sp.astype(bf), wk_sp.astype(bf), wv_t.astype(bf)


def _emat() -> np.ndarray:
    """One-hot row maps for the final-pair epilogue matmuls, four 128x128
    lhsT blocks [E0_d | E1_d | E0_v | E1_v]:
    Ed_X[g*32+16, X*64+g*16+r] = 1 for r<16 (denominator broadcast)
    Ev_X[g*32+r,  X*64+g*16+r] = 1          (value row permute)
    The X=0/1 matmuls accumulate into one PSUM bank, each filling its own
    64-partition half."""
    e = np.zeros((128, 512), np.float32)
    for X in range(2):
        for g in range(4):
            for r in range(DH):
                e[g * 32 + DH, X * 128 + X * 64 + g * 16 + r] = 1.0
                e[g * 32 + r, 256 + X * 128 + X * 64 + g * 16 + r] = 1.0
    return e


COMPACT_KV = True  # drop masked kv positions host-side (exact: they contribute
                   # exp(-1e30)=0 to softmax numerator and denominator alike)


def prepare(queries: np.ndarray, mask: np.ndarray, w_mem: np.ndarray,
            w_query: np.ndarray):
    """Build (compiled program, per-core input maps)."""
    assert queries.shape == (B, D, L) and mask.shape == (B, L)
    maskf = mask.astype(np.float32)
    kept = [np.nonzero(maskf[b] > 0.0)[0] for b in range(B)]
    if COMPACT_KV and all(len(k) > 0 for k in kept):
        n_kv = max(1, -(-max(len(k) for k in kept) // 128))
    else:
        n_kv = L // 128
        kept = None
    Lkv = n_kv * 128

    nc = _program_cache.get(n_kv)
    if nc is None:
        nc = _program_cache[n_kv] = _build(n_kv)

    wq_sp, wk_sp, wv_t = _prep_weights(
        w_mem.astype(np.float32), w_query.astype(np.float32))
    em = _emat()

    bf = ml_dtypes.bfloat16
    in_maps = []
    for b in range(B):
        qb = np.ascontiguousarray(queries[b], dtype=np.float32)
        if kept is not None:
            idx = kept[b]
            qkv = np.zeros((D, Lkv), np.float32)
            qkv[:, :len(idx)] = qb[:, idx]
            val = np.zeros(Lkv, np.float32)
            val[:len(idx)] = 1.0
        else:
            qkv = qb
            val = maskf[b]
        in_maps.append({
            "q_full": np.ascontiguousarray(qb.astype(bf)),
            "q_kv": np.ascontiguousarray(qkv.astype(bf)),
            "wq_sp": wq_sp,
            "wk_sp": wk_sp,
            "wv_t": wv_t,
            "valid": np.ascontiguousarray(val.reshape(n_kv, 128).T),
            "emat": em,
        })
    return nc, in_maps


def kernel(queries: np.ndarray, mask: np.ndarray, w_mem: np.ndarray,
           w_query: np.ndarray) -> np.ndarray:
    nc, in_maps = prepare(queries, mask, w_mem, w_query)
    res = bass_utils.run_bass_kernel_spmd(nc, in_maps, core_ids=list(range(B)))
    return np.stack([res.results[b]["out"] for b in range(B)]).astype(np.float32)

